# revision 15
# baseline (speedup 1.0000x reference)
"""Fused single-launch BPCA pooling v5 (bf16 data plane).

Per core: 4 samples. Per sample:
  - DMA 8 half-chunks [128, 2048] f32 into f32 staging; gpsimd casts each
    half to a bf16 chunk tile [128, 4096].
  - Gram G128 in bf16 on the PE (32 matmuls/chunk, 128-wide moving),
    accumulated in one PSUM tile [128, 128].
  - Extraction: mask-mult + strided reduce -> [128,4], PE-fold to S [4,4],
    spread to replicated Sflat [128, 16] (all tables from aux).
  - Top eigenvector: Gershgorin-normalized power iteration; 7 squarings
    as [4,4] fp32 PE matmuls with scalar-engine PSUM->SBUF copies
    (eigenvalues normalized into [~0.5, 1] so no renorms are needed).
  - Projection y = x . v on the PE: weights v_k*I128 (bf16, built by one
    tensor_scalar each), 4 matmuls per half-chunk accumulating the four
    stride-4 planes into PSUM [128, 512]; vector/scalar copy to SBUF,
    DMA out. Mean-centering, 1/||v|| and the LAPACK sign fold into one
    host-side post-scale using the returned stats.

DMA queue discipline: the sync queue carries ONLY input DMAs (so input
streaming is never blocked); output DMAs and the tiny eigen transfer
are issued from the scalar queue.
"""

import numpy as np
from contextlib import ExitStack

import concourse.bass as bass
import concourse.tile as tile
from concourse import bacc, mybir
from concourse.bass_utils import run_bass_kernel_spmd

B, H, W, C = 32, 64, 64, 512
N_CORES = 8
BPC = B // N_CORES
SAMPLE = H * W * C
NROWS = SAMPLE // 4
OUT_SAMPLE = SAMPLE // 4
F32 = mybir.dt.float32
BF16 = mybir.dt.bfloat16
ALU = mybir.AluOpType
AF = mybir.ActivationFunctionType
AXL = mybir.AxisListType

NSQ = 7                       # squarings; worst contamination ~3e-5
EVEC = [0.9129, -0.6011, 0.3683, 1.0577]   # fixed generic seed vector

# aux column layout
MMF_OFF = 0                   # 16 per sample: flat mu mu^T
C_E = 16 * BPC                # 1 col: eigen seed, rows 0..3
C_I4 = C_E + 1                # 4 cols: I4, rows 0..3
C_FM = C_I4 + 4               # 16 cols: FM[p,u] = (u//4 == p)
C_DM = C_FM + 16              # 16 cols: dm16 flat identity
C_DMQ = C_DM + 16             # 16 cols: 0.25*dm16
C_E4 = C_DMQ + 16             # 4 cols: E4[p,k] = (p%4 == k)
C_ONES = C_E4 + 4             # 128 cols: ones
C_M = C_ONES + 128            # 128 cols: mask (n>>2 == p>>2)
C_I128 = C_M + 128            # 128 cols: I128
AUXW = C_I128 + 128


def _in_dram_ap_half(x, b, half, q, h2):
    off = b * SAMPLE + half * 32768 + q * 4096 + h2 * 2048
    return bass.AP(x, off, [[65536, 32], [8192, 4], [1, 2048]])


def _v(ap, axes, extra_off=0):
    """Free-dim view of a [P, F] tile AP with custom free axes."""
    return bass.AP(ap.tensor, ap.offset + extra_off, [list(ap.ap[0])] + axes)


def _build_fused():
    nc = bacc.Bacc("TRN2", target_bir_lowering=False, debug=False)
    x = nc.dram_tensor("x", [BPC * SAMPLE], F32, kind="ExternalInput")
    aux = nc.dram_tensor("aux", [128, AUXW], F32, kind="ExternalInput")
    y = nc.dram_tensor("y", [BPC * OUT_SAMPLE], F32, kind="ExternalOutput")
    st = nc.dram_tensor("stats", [1, BPC * 20], F32, kind="ExternalOutput")

    with tile.TileContext(nc) as tc, ExitStack() as ctx:
        const = ctx.enter_context(tc.tile_pool(name="const", bufs=1))
        stag = ctx.enter_context(tc.tile_pool(name="stag", bufs=3))
        chunks = ctx.enter_context(tc.tile_pool(name="chunks", bufs=12))
        psumg = ctx.enter_context(tc.tile_pool(name="psumg", bufs=2, space="PSUM"))
        outpp = ctx.enter_context(tc.tile_pool(name="outpp", bufs=3, space="PSUM"))
        psums = ctx.enter_context(tc.tile_pool(name="psums", bufs=1, space="PSUM"))
        red = ctx.enter_context(tc.tile_pool(name="red", bufs=2))
        eig = ctx.enter_context(tc.tile_pool(name="eig", bufs=2))
        plp = ctx.enter_context(tc.tile_pool(name="plp", bufs=3))

        auxp = const.tile([128, AUXW], F32)
        nc.sync.dma_start(auxp[:], bass.AP(aux, 0, [[AUXW, 128], [1, AUXW]]))
        sttile = const.tile([1, BPC * 20], F32)

        def emit_gram_half(bt, h2, psg, first, last):
            for i in range(16):
                j = h2 * 16 + i
                sl = bt[:, j * 128:(j + 1) * 128]
                nc.tensor.matmul(psg[:], sl, sl, start=(first and i == 0),
                                 stop=(last and i == 15))

        def emit_extract_eigen(b, psg):
            mask = auxp[:, C_M:C_M + 128]
            dm16 = auxp[:, C_DM:C_DM + 16]
            dm16q = auxp[:, C_DMQ:C_DMQ + 16]
            # ---- extraction: S = sum_g G128[4g+k, 4g+l] ----
            mA = red.tile([128, 128], F32, tag="mA")
            nc.vector.tensor_mul(mA[:], psg[:], mask)
            m4A = red.tile([128, 4], F32, tag="m4A")
            nc.vector.tensor_reduce(m4A[:], _v(mA[:], [[1, 4], [4, 32]]),
                                    AXL.X, ALU.add)
            psE = psums.tile([4, 4], F32, tag="scr", name=f"psE_{b}")
            nc.tensor.matmul(psE[:], auxp[:, C_E4:C_E4 + 4], m4A[:],
                             start=True, stop=True)
            Fm16 = red.tile([4, 16], F32, tag="Fm16")
            s_b = _v(psE[:], [[0, 4], [1, 4]])
            nc.vector.tensor_tensor(Fm16[:].rearrange("p (j l) -> p j l", j=4),
                                    s_b,
                                    _v(auxp[0:4, :], [[4, 4], [1, 4]], C_FM),
                                    ALU.mult)
            psS = psums.tile([128, 16], F32, tag="psS", name=f"psS_{b}")
            nc.tensor.matmul(psS[:], auxp[0:4, C_ONES:C_ONES + 128], Fm16[:],
                             start=True, stop=True)

            # ---- eigen setup (replicated flat [128, 16]) ----
            covf = eig.tile([128, 16], F32, tag="covf")
            nc.vector.scalar_tensor_tensor(
                covf[:], psS[:], 1.0 / NROWS,
                auxp[:, MMF_OFF + 16 * b:MMF_OFF + 16 * b + 16],
                ALU.mult, ALU.subtract)
            trqn = eig.tile([128, 1], F32, tag="trqn")
            nc.vector.tensor_reduce(trqn[:], _v(covf[:], [[5, 4]]), AXL.X,
                                    ALU.add, negate=True)
            B0 = eig.tile([128, 16], F32, tag="B0")
            nc.vector.scalar_tensor_tensor(B0[:], dm16q, trqn[:], covf[:],
                                           ALU.mult, ALU.add)
            absr = eig.tile([128, 4], F32, tag="absr")
            nc.vector.tensor_reduce(absr[:].rearrange("p (i u) -> p i u", i=4),
                                    B0[:].rearrange("p (i j) -> p i j", i=4),
                                    AXL.X, ALU.add, apply_absolute_value=True)
            rsh = eig.tile([128, 1], F32, tag="rsh")
            nc.vector.tensor_reduce(rsh[:], absr[:], AXL.X, ALU.max)
            rrec = eig.tile([128, 1], F32, tag="rrec")
            nc.vector.reciprocal(rrec[:], rsh[:])
            Bc = eig.tile([128, 16], F32, tag="Bc")
            nc.vector.scalar_tensor_tensor(Bc[:], dm16, rsh[:], B0[:],
                                           ALU.mult, ALU.add)
            nc.vector.tensor_scalar(Bc[:], Bc[:], rrec[:], 0.5, ALU.mult,
                                    ALU.mult)
            # ---- squarings as [4,4] PE matmuls ----
            Bpe = eig.tile([4, 4], F32, tag="Bpe")
            nc.scalar.dma_start(Bpe[:], _v(Bc[0:1, :], [[1, 16]]))
            cur = Bpe
            for k in range(NSQ):
                psq = psums.tile([4, 4], F32, tag="scr", name=f"psq_{b}_{k}")
                nc.tensor.matmul(psq[:], cur[:], cur[:], start=True, stop=True)
                nxt = eig.tile([4, 4], F32, tag=f"sq{k % 2}")
                nc.scalar.copy(nxt[:], psq[:])
                cur = nxt
            # ---- v = C @ e, replicate across partitions via PE ----
            vps = psums.tile([4, 1], F32, tag="scr", name=f"vps_{b}")
            nc.tensor.matmul(vps[:], cur[:], auxp[0:4, C_E:C_E + 1],
                             start=True, stop=True)
            vdiag = eig.tile([4, 4], F32, tag="vdiag")
            nc.vector.tensor_tensor(vdiag[:], _v(vps[:], [[0, 4]]),
                                    auxp[0:4, C_I4:C_I4 + 4], ALU.mult)
            vrps = psums.tile([128, 4], F32, tag="scr", name=f"vrps_{b}")
            nc.tensor.matmul(vrps[:], auxp[0:4, C_ONES:C_ONES + 128],
                             vdiag[:], start=True, stop=True)
            v_rep = eig.tile([128, 4], F32, tag="v_rep")
            nc.scalar.copy(v_rep[:], vrps[:])
            nc.scalar.copy(sttile[:, 20 * b:20 * b + 16], psS[0:1, :])
            nc.scalar.copy(sttile[:, 20 * b + 16:20 * b + 20], v_rep[0:1, :])
            wks = []
            for k in range(4):
                wk = eig.tile([128, 128], BF16, tag=f"wk{k}", name=f"wk{b}_{k}")
                nc.vector.tensor_scalar(wk[:], auxp[:, C_I128:C_I128 + 128],
                                        v_rep[:, k:k + 1], None, ALU.mult)
                wks.append(wk)
            return wks

        def emit_proj(pb, pctiles, wks):
            for ci, bt in enumerate(pctiles):
                half, q = divmod(ci, 2)
                pa = plp.tile([128, 1024], F32, tag="pa", name=f"pa{pb}_{ci}")
                for h in range(2):
                    outp = outpp.tile([128, 512], F32, tag="outp",
                                      name=f"op{pb}_{ci}_{h}")
                    for k in range(4):
                        rhs = _v(bt[:], [[512, 4], [4, 128]],
                                 extra_off=k + h * 2048)
                        nc.tensor.matmul(outp[:], wks[k][:], rhs,
                                         start=(k == 0), stop=(k == 3))
                    eng = nc.vector if h == 0 else nc.scalar
                    if h == 0:
                        nc.vector.tensor_copy(pa[:, 0:512], outp[:])
                    else:
                        nc.scalar.copy(pa[:, 512:1024], outp[:])
                nc.scalar.dma_start(
                    bass.AP(y, pb * OUT_SAMPLE + q * 2048 + half * 256,
                            [[4096, 128], [512, 4], [1, 256]]),
                    pa[:])

        prev = None
        for b in range(BPC):
            btiles = []
            psg = psumg.tile([128, 128], F32, tag="psg", name=f"psg_{b}")
            casts = []
            for ci in range(4):
                half, q = divmod(ci, 2)
                ft = stag.tile([128, 4096], F32, tag="stg", name=f"f_{b}_{ci}")
                bt = chunks.tile([128, 4096], BF16, tag="chunk",
                                 name=f"t_{b}_{ci}")
                btiles.append(bt)
                for h2 in range(2):
                    nc.sync.dma_start(ft[:, h2 * 2048:(h2 + 1) * 2048],
                                      _in_dram_ap_half(x, b, half, q, h2))
                    nc.gpsimd.tensor_copy(bt[:, h2 * 2048:(h2 + 1) * 2048],
                                          ft[:, h2 * 2048:(h2 + 1) * 2048])
                    casts.append((bt, h2))
            if prev is not None:
                emit_proj(*prev)
            for ci in range(4):
                for h2 in range(2):
                    emit_gram_half(btiles[ci], h2, psg,
                                   first=(ci == 0 and h2 == 0),
                                   last=(ci == 3 and h2 == 1))
            wks = emit_extract_eigen(b, psg)
            prev = (b, btiles, wks)
        emit_proj(*prev)
        nc.scalar.dma_start(bass.AP(st, 0, [[BPC * 20, 1], [1, BPC * 20]]),
                            sttile[:])
    nc.compile()
    return nc


_CACHE = {}


def _get(name, builder):
    if name not in _CACHE:
        _CACHE[name] = builder()
    return _CACHE[name]


def make_aux(mean):
    """mean: [BPC, 4] float -> aux array [128, AUXW]."""
    a = np.zeros((128, AUXW), np.float32)
    p = np.arange(128)
    for b in range(BPC):
        mm = np.outer(mean[b], mean[b]).astype(np.float32).reshape(16)
        a[:, MMF_OFF + 16 * b:MMF_OFF + 16 * b + 16] = mm
    a[0:4, C_E] = np.asarray(EVEC, np.float32)
    a[0:4, C_I4:C_I4 + 4] = np.eye(4, dtype=np.float32)
    u = np.arange(16)
    a[0:4, C_FM:C_FM + 16] = (u[None, :] // 4 == np.arange(4)[:, None])
    a[:, C_DM:C_DM + 16] = ((u // 4) == (u % 4)).astype(np.float32)[None, :]
    a[:, C_DMQ:C_DMQ + 16] = 0.25 * a[:, C_DM:C_DM + 16]
    a[:, C_E4:C_E4 + 4] = (np.arange(4)[None, :] == (p % 4)[:, None])
    a[:, C_ONES:C_ONES + 128] = 1.0
    n = np.arange(128)
    a[:, C_M:C_M + 128] = ((n[None, :] >> 2) == (p >> 2)[:, None])
    a[:, C_I128:C_I128 + 128] = np.eye(128, dtype=np.float32)
    return a


def kernel(inputs: np.ndarray) -> np.ndarray:
    xx = np.ascontiguousarray(np.asarray(inputs, dtype=np.float32))
    assert xx.shape == (B, H, W, C), xx.shape
    xf = xx.reshape(N_CORES, BPC * SAMPLE)
    cores = list(range(N_CORES))
    mean = xx.reshape(B, NROWS, 4).mean(axis=1, dtype=np.float64)  # [B, 4]

    nc = _get("fused", _build_fused)
    in_maps = [
        {"x": xf[c], "aux": make_aux(mean[c * BPC:(c + 1) * BPC])} for c in cores
    ]
    r = run_bass_kernel_spmd(nc, in_maps, cores)
    stats = np.stack([r.results[c]["stats"] for c in cores]).reshape(B, 20)
    yv = np.stack([r.results[c]["y"] for c in cores]).reshape(B, OUT_SAMPLE)

    S = stats[:, 0:16].reshape(B, 4, 4).astype(np.float64)
    v_dev = stats[:, 16:20].astype(np.float64)
    cov = (S / NROWS - np.einsum("bi,bj->bij", mean, mean)).astype(np.float32)

    import jax
    import jax.numpy as jnp
    with jax.default_device(jax.devices("cpu")[0]):
        _, vecs = jnp.linalg.eigh(jnp.asarray(cov))
    v_ref = np.asarray(vecs)[:, :, -1].astype(np.float64)

    # the device projected with bf16(v_dev) weights: use those exact values
    import ml_dtypes
    v_bf = v_dev.astype(np.float32).astype(ml_dtypes.bfloat16).astype(
        np.float64)
    dot = (v_ref * v_dev).sum(1)
    scale = np.sign(dot) / np.linalg.norm(v_bf, axis=1)
    offs = -(mean * v_bf).sum(1) * scale          # fold -mu.v into host
    yv = (yv * scale[:, None] + offs[:, None]).astype(np.float32)
    return yv.reshape(B, H // 2, W // 2, C)


# revision 17
# speedup vs baseline: 1.5069x; 1.5069x over previous
"""Fused single-launch BPCA pooling v5 (bf16 data plane).

Per core: 4 samples. Per sample:
  - DMA 8 half-chunks [128, 2048] f32 into f32 staging; gpsimd casts each
    half to a bf16 chunk tile [128, 4096].
  - Gram G128 in bf16 on the PE (32 matmuls/chunk, 128-wide moving),
    accumulated in one PSUM tile [128, 128].
  - Extraction: mask-mult + strided reduce -> [128,4], PE-fold to S [4,4],
    spread to replicated Sflat [128, 16] (all tables from aux).
  - Top eigenvector: Gershgorin-normalized power iteration; 7 squarings
    as [4,4] fp32 PE matmuls with scalar-engine PSUM->SBUF copies
    (eigenvalues normalized into [~0.5, 1] so no renorms are needed).
  - Projection y = x . v on the PE: weights v_k*I128 (bf16, built by one
    tensor_scalar each), 4 matmuls per half-chunk accumulating the four
    stride-4 planes into PSUM [128, 512]; vector/scalar copy to SBUF,
    DMA out. Mean-centering, 1/||v|| and the LAPACK sign fold into one
    host-side post-scale using the returned stats.

DMA queue discipline: the sync queue carries ONLY input DMAs (so input
streaming is never blocked); output DMAs and the tiny eigen transfer
are issued from the scalar queue.
"""

import numpy as np
from contextlib import ExitStack

import concourse.bass as bass
import concourse.tile as tile
from concourse import bacc, mybir
from concourse.bass_utils import run_bass_kernel_spmd

B, H, W, C = 32, 64, 64, 512
N_CORES = 8
BPC = B // N_CORES
SAMPLE = H * W * C
NROWS = SAMPLE // 4
OUT_SAMPLE = SAMPLE // 4
F32 = mybir.dt.float32
BF16 = mybir.dt.bfloat16
ALU = mybir.AluOpType
AF = mybir.ActivationFunctionType
AXL = mybir.AxisListType

NSQ = 7                       # squarings; worst contamination ~3e-5
EVEC = [0.9129, -0.6011, 0.3683, 1.0577]   # fixed generic seed vector

# aux column layout
MMF_OFF = 0                   # 16 per sample: flat mu mu^T
C_E = 16 * BPC                # 1 col: eigen seed, rows 0..3
C_I4 = C_E + 1                # 4 cols: I4, rows 0..3
C_FM = C_I4 + 4               # 16 cols: FM[p,u] = (u//4 == p)
C_DM = C_FM + 16              # 16 cols: dm16 flat identity
C_DMQ = C_DM + 16             # 16 cols: 0.25*dm16
C_E4 = C_DMQ + 16             # 4 cols: E4[p,k] = (p%4 == k)
C_ONES = C_E4 + 4             # 128 cols: ones
C_M = C_ONES + 128            # 128 cols: mask (n>>2 == p>>2)
C_I128 = C_M + 128            # 128 cols: I128
AUXW = C_I128 + 128


def _in_dram_ap_half(x, b, half, q, h2):
    off = b * SAMPLE + half * 32768 + q * 4096 + h2 * 2048
    return bass.AP(x, off, [[65536, 32], [8192, 4], [1, 2048]])


def _v(ap, axes, extra_off=0):
    """Free-dim view of a [P, F] tile AP with custom free axes."""
    return bass.AP(ap.tensor, ap.offset + extra_off, [list(ap.ap[0])] + axes)


def _build_fused():
    nc = bacc.Bacc("TRN2", target_bir_lowering=False, debug=False)
    x = nc.dram_tensor("x", [BPC * SAMPLE], F32, kind="ExternalInput")
    aux = nc.dram_tensor("aux", [128, AUXW], F32, kind="ExternalInput")
    y = nc.dram_tensor("y", [BPC * OUT_SAMPLE], F32, kind="ExternalOutput")
    st = nc.dram_tensor("stats", [1, BPC * 20], F32, kind="ExternalOutput")

    with tile.TileContext(nc) as tc, ExitStack() as ctx:
        const = ctx.enter_context(tc.tile_pool(name="const", bufs=1))
        stag = ctx.enter_context(tc.tile_pool(name="stag", bufs=3))
        chunks = ctx.enter_context(tc.tile_pool(name="chunks", bufs=12))
        psumg = ctx.enter_context(tc.tile_pool(name="psumg", bufs=2, space="PSUM"))
        outpp = ctx.enter_context(tc.tile_pool(name="outpp", bufs=3, space="PSUM"))
        psums = ctx.enter_context(tc.tile_pool(name="psums", bufs=1, space="PSUM"))
        red = ctx.enter_context(tc.tile_pool(name="red", bufs=2))
        eig = ctx.enter_context(tc.tile_pool(name="eig", bufs=2))
        plp = ctx.enter_context(tc.tile_pool(name="plp", bufs=3))

        auxp = const.tile([128, AUXW], F32)
        nc.sync.dma_start(auxp[:], bass.AP(aux, 0, [[AUXW, 128], [1, AUXW]]))
        sttile = const.tile([1, BPC * 20], F32)

        def emit_gram_half(bt, h2, psg, first, last):
            for i in range(16):
                j = h2 * 16 + i
                sl = bt[:, j * 128:(j + 1) * 128]
                nc.tensor.matmul(psg[:], sl, sl, start=(first and i == 0),
                                 stop=(last and i == 15))

        def emit_extract_eigen(b, psg):
            mask = auxp[:, C_M:C_M + 128]
            dm16 = auxp[:, C_DM:C_DM + 16]
            dm16q = auxp[:, C_DMQ:C_DMQ + 16]
            # ---- extraction: S = sum_g G128[4g+k, 4g+l] ----
            mA = red.tile([128, 128], F32, tag="mA")
            nc.vector.tensor_mul(mA[:], psg[:], mask)
            m4A = red.tile([128, 4], F32, tag="m4A")
            nc.vector.tensor_reduce(m4A[:], _v(mA[:], [[1, 4], [4, 32]]),
                                    AXL.X, ALU.add)
            psE = psums.tile([4, 4], F32, tag="scr", name=f"psE_{b}")
            nc.tensor.matmul(psE[:], auxp[:, C_E4:C_E4 + 4], m4A[:],
                             start=True, stop=True)
            Fm16 = red.tile([4, 16], F32, tag="Fm16")
            s_b = _v(psE[:], [[0, 4], [1, 4]])
            nc.vector.tensor_tensor(Fm16[:].rearrange("p (j l) -> p j l", j=4),
                                    s_b,
                                    _v(auxp[0:4, :], [[4, 4], [1, 4]], C_FM),
                                    ALU.mult)
            psS = psums.tile([128, 16], F32, tag="psS", name=f"psS_{b}")
            nc.tensor.matmul(psS[:], auxp[0:4, C_ONES:C_ONES + 128], Fm16[:],
                             start=True, stop=True)

            # ---- eigen setup (replicated flat [128, 16]) ----
            covf = eig.tile([128, 16], F32, tag="covf")
            nc.vector.scalar_tensor_tensor(
                covf[:], psS[:], 1.0 / NROWS,
                auxp[:, MMF_OFF + 16 * b:MMF_OFF + 16 * b + 16],
                ALU.mult, ALU.subtract)
            trqn = eig.tile([128, 1], F32, tag="trqn")
            nc.vector.tensor_reduce(trqn[:], _v(covf[:], [[5, 4]]), AXL.X,
                                    ALU.add, negate=True)
            B0 = eig.tile([128, 16], F32, tag="B0")
            nc.vector.scalar_tensor_tensor(B0[:], dm16q, trqn[:], covf[:],
                                           ALU.mult, ALU.add)
            absr = eig.tile([128, 4], F32, tag="absr")
            nc.vector.tensor_reduce(absr[:].rearrange("p (i u) -> p i u", i=4),
                                    B0[:].rearrange("p (i j) -> p i j", i=4),
                                    AXL.X, ALU.add, apply_absolute_value=True)
            rsh = eig.tile([128, 1], F32, tag="rsh")
            nc.vector.tensor_reduce(rsh[:], absr[:], AXL.X, ALU.max)
            rrec = eig.tile([128, 1], F32, tag="rrec")
            nc.vector.reciprocal(rrec[:], rsh[:])
            Bc = eig.tile([128, 16], F32, tag="Bc")
            nc.vector.scalar_tensor_tensor(Bc[:], dm16, rsh[:], B0[:],
                                           ALU.mult, ALU.add)
            nc.vector.tensor_scalar(Bc[:], Bc[:], rrec[:], 0.5, ALU.mult,
                                    ALU.mult)
            # ---- squarings as [4,4] PE matmuls ----
            Bpe = eig.tile([4, 4], F32, tag="Bpe")
            nc.scalar.dma_start(Bpe[:], _v(Bc[0:1, :], [[1, 16]]))
            cur = Bpe
            for k in range(NSQ):
                psq = psums.tile([4, 4], F32, tag="scr", name=f"psq_{b}_{k}")
                nc.tensor.matmul(psq[:], cur[:], cur[:], start=True, stop=True)
                nxt = eig.tile([4, 4], F32, tag=f"sq{k % 2}")
                nc.scalar.copy(nxt[:], psq[:])
                cur = nxt
            # ---- v = C @ e, replicate across partitions via PE ----
            vps = psums.tile([4, 1], F32, tag="scr", name=f"vps_{b}")
            nc.tensor.matmul(vps[:], cur[:], auxp[0:4, C_E:C_E + 1],
                             start=True, stop=True)
            vdiag = eig.tile([4, 4], F32, tag="vdiag")
            nc.vector.tensor_tensor(vdiag[:], _v(vps[:], [[0, 4]]),
                                    auxp[0:4, C_I4:C_I4 + 4], ALU.mult)
            vrps = psums.tile([128, 4], F32, tag="scr", name=f"vrps_{b}")
            nc.tensor.matmul(vrps[:], auxp[0:4, C_ONES:C_ONES + 128],
                             vdiag[:], start=True, stop=True)
            v_rep = eig.tile([128, 4], F32, tag="v_rep")
            nc.scalar.copy(v_rep[:], vrps[:])
            nc.scalar.copy(sttile[:, 20 * b:20 * b + 16], psS[0:1, :])
            nc.scalar.copy(sttile[:, 20 * b + 16:20 * b + 20], v_rep[0:1, :])
            wks = []
            for k in range(4):
                wk = eig.tile([128, 128], BF16, tag=f"wk{k}", name=f"wk{b}_{k}")
                nc.vector.tensor_scalar(wk[:], auxp[:, C_I128:C_I128 + 128],
                                        v_rep[:, k:k + 1], None, ALU.mult)
                wks.append(wk)
            return wks

        def emit_proj(pb, pctiles, wks):
            for ci, bt in enumerate(pctiles):
                half, q = divmod(ci, 2)
                pa = plp.tile([128, 1024], F32, tag="pa", name=f"pa{pb}_{ci}")
                for h in range(2):
                    outp = outpp.tile([128, 512], F32, tag="outp",
                                      name=f"op{pb}_{ci}_{h}")
                    for k in range(4):
                        rhs = _v(bt[:], [[512, 4], [4, 128]],
                                 extra_off=k + h * 2048)
                        nc.tensor.matmul(outp[:], wks[k][:], rhs,
                                         start=(k == 0), stop=(k == 3))
                    if h == 0:
                        nc.vector.tensor_copy(pa[:, 0:512], outp[:])
                    else:
                        nc.scalar.copy(pa[:, 512:1024], outp[:])
                nc.scalar.dma_start(
                    bass.AP(y, pb * OUT_SAMPLE + q * 2048 + half * 256,
                            [[4096, 128], [512, 4], [1, 256]]),
                    pa[:])

        prev = None
        for b in range(BPC):
            btiles = []
            psg = psumg.tile([128, 128], F32, tag="psg", name=f"psg_{b}")
            casts = []
            for ci in range(4):
                half, q = divmod(ci, 2)
                ft = stag.tile([128, 4096], F32, tag="stg", name=f"f_{b}_{ci}")
                bt = chunks.tile([128, 4096], BF16, tag="chunk",
                                 name=f"t_{b}_{ci}")
                btiles.append(bt)
                for h2 in range(2):
                    hidx = ci * 2 + h2
                    nc.sync.dma_start(ft[:, h2 * 2048:(h2 + 1) * 2048],
                                      _in_dram_ap_half(x, b, half, q, h2))
                    dst = bt[:, h2 * 2048:(h2 + 1) * 2048]
                    srcv = ft[:, h2 * 2048:(h2 + 1) * 2048]
                    if hidx == 3:
                        nc.gpsimd.tensor_copy(dst, srcv)
                    elif hidx in (1, 5):
                        nc.scalar.copy(dst, srcv)
                    else:
                        nc.vector.tensor_copy(dst, srcv)
                    casts.append((bt, h2))
            if prev is not None:
                emit_proj(*prev)
            for ci in range(4):
                for h2 in range(2):
                    emit_gram_half(btiles[ci], h2, psg,
                                   first=(ci == 0 and h2 == 0),
                                   last=(ci == 3 and h2 == 1))
            wks = emit_extract_eigen(b, psg)
            prev = (b, btiles, wks)
        emit_proj(*prev)
        nc.scalar.dma_start(bass.AP(st, 0, [[BPC * 20, 1], [1, BPC * 20]]),
                            sttile[:])
    nc.compile()
    return nc


_CACHE = {}


def _get(name, builder):
    if name not in _CACHE:
        _CACHE[name] = builder()
    return _CACHE[name]


def make_aux(mean):
    """mean: [BPC, 4] float -> aux array [128, AUXW]."""
    a = np.zeros((128, AUXW), np.float32)
    p = np.arange(128)
    for b in range(BPC):
        mm = np.outer(mean[b], mean[b]).astype(np.float32).reshape(16)
        a[:, MMF_OFF + 16 * b:MMF_OFF + 16 * b + 16] = mm
    a[0:4, C_E] = np.asarray(EVEC, np.float32)
    a[0:4, C_I4:C_I4 + 4] = np.eye(4, dtype=np.float32)
    u = np.arange(16)
    a[0:4, C_FM:C_FM + 16] = (u[None, :] // 4 == np.arange(4)[:, None])
    a[:, C_DM:C_DM + 16] = ((u // 4) == (u % 4)).astype(np.float32)[None, :]
    a[:, C_DMQ:C_DMQ + 16] = 0.25 * a[:, C_DM:C_DM + 16]
    a[:, C_E4:C_E4 + 4] = (np.arange(4)[None, :] == (p % 4)[:, None])
    a[:, C_ONES:C_ONES + 128] = 1.0
    n = np.arange(128)
    a[:, C_M:C_M + 128] = ((n[None, :] >> 2) == (p >> 2)[:, None])
    a[:, C_I128:C_I128 + 128] = np.eye(128, dtype=np.float32)
    return a


def kernel(inputs: np.ndarray) -> np.ndarray:
    xx = np.ascontiguousarray(np.asarray(inputs, dtype=np.float32))
    assert xx.shape == (B, H, W, C), xx.shape
    xf = xx.reshape(N_CORES, BPC * SAMPLE)
    cores = list(range(N_CORES))
    mean = xx.reshape(B, NROWS, 4).mean(axis=1, dtype=np.float64)  # [B, 4]

    nc = _get("fused", _build_fused)
    in_maps = [
        {"x": xf[c], "aux": make_aux(mean[c * BPC:(c + 1) * BPC])} for c in cores
    ]
    r = run_bass_kernel_spmd(nc, in_maps, cores)
    stats = np.stack([r.results[c]["stats"] for c in cores]).reshape(B, 20)
    yv = np.stack([r.results[c]["y"] for c in cores]).reshape(B, OUT_SAMPLE)

    S = stats[:, 0:16].reshape(B, 4, 4).astype(np.float64)
    v_dev = stats[:, 16:20].astype(np.float64)
    cov = (S / NROWS - np.einsum("bi,bj->bij", mean, mean)).astype(np.float32)

    import jax
    import jax.numpy as jnp
    with jax.default_device(jax.devices("cpu")[0]):
        _, vecs = jnp.linalg.eigh(jnp.asarray(cov))
    v_ref = np.asarray(vecs)[:, :, -1].astype(np.float64)

    # the device projected with bf16(v_dev) weights: use those exact values
    import ml_dtypes
    v_bf = v_dev.astype(np.float32).astype(ml_dtypes.bfloat16).astype(
        np.float64)
    dot = (v_ref * v_dev).sum(1)
    scale = np.sign(dot) / np.linalg.norm(v_bf, axis=1)
    offs = -(mean * v_bf).sum(1) * scale          # fold -mu.v into host
    yv = (yv * scale[:, None] + offs[:, None]).astype(np.float32)
    return yv.reshape(B, H // 2, W // 2, C)
